# revision 6
# baseline (speedup 1.0000x reference)
"""MoE (top-1 routed) Trainium2 kernel.

Strategy: the reference computes every expert for every token and then
selects one expert per token with a one-hot gate.  Mathematically the
output for token n is expert_out[argmax_e logits[n, e], n], so we compute
the gating on host (bitwise-matching the reference's fp32 `x @ Wg + bg`
on CPU), group tokens by their selected expert, and run expert e's
pipeline for only its own tokens on NeuronCore e (expert-parallel, an
all-reduce-free gather).  This is 8x less device compute than the dense
reference formulation.

Device pipeline per core (C = padded token count, transposed layout with
features on partitions and tokens on the free dim):
    h^T[u, n]  = W1^T x^T          (PE, K=1024 accumulated in PSUM)
    sw         = (tanh(h/2) + 1) * h            # == 2*swish(h)
    z^T[v, n]  = (0.5*proj)^T sw   (PE)         # 0.5 folds the 2 above
    t2         = tanh(z/2)                      # == 2*sigmoid(z) - 1
    q          = exp((32/7) * t2)               # ONE exp per block
      -- the reference's gaussian basis times exp(32*xn^2) is
         g_j = exp(32*k_j*t2 + 32*k_j*(1-k_j)) = c_j * q^j  (k_j = j/7),
         so the basis numerator/denominator are degree-7 polynomials in
         q.  The per-element factor cancels in the normalization (the
         reference's +1e-6 in the denominator is a <=1.2e-6 relative
         perturbation, below fp32 matmul noise).
    powers q^2..q^7 via ACT square + DVE/GPSIMD multiplies (bf16)
    num        = sum_j (cv_j*c_j) q^j   (PE; cv = ctrl * scaling)
    den        = 1 + sum_j c_j q^j      (PE; the 1 via a ones tile)
    out^T[u,n] = (num + cv_0) * reciprocal(den)

tanh/exp/square share one ACT table set ("exp_and_others"), so the
scalar engine never pays the ~2.7us table switch.

num/den accumulate on the PE as diagonal matmuls.  A 128x128 diagonal
weight only occupies the 4 diagonal 32x32 subarrays of the PE, so in
packed mode ("tile4") each accumulation step issues 16 concurrent 32x32
tile_position matmuls: 4 chains (num or den of the 4 unit blocks vc)
at rotations rot=vc, chain vc at positions (32r, 32((r+vc)%4)), each
row-group r consuming that chain's power tile slice [32r:32r+32).
Sub-128 dst partitions are only legal for 16-bit dtypes, hence the
basis runs in bf16 (host-simulated rel err 4.8e-3 vs the 2e-2 budget;
the main matmul path stays f32r).  Chain outputs are partition-block-
rotated by vc; a0 is pre-rotated host-side and the output DMA
unscrambles (2 descriptors per vc).
"""

import os
from contextlib import ExitStack

import numpy as np

N_TOK, D_IN, U_DIM, E_EXP, B_BAS = 8192, 1024, 512, 8, 8
N_CORES = 8
P = 128
TNMAX = 512

PACK_MODE = os.environ.get("MOE_PACK", "tile4")  # "plain" | "tile4"
SQ_ACT = int(os.environ.get("MOE_SQ_ACT", "2"))  # q2/q4 via ACT square (0-2)
N_PW_DVE = int(os.environ.get("MOE_PW_DVE", "2"))  # TT powers on DVE (rest GPSIMD)
G_BUFS = int(os.environ.get("MOE_GBUFS", "32"))
X_BUFS = int(os.environ.get("MOE_XBUFS", "2"))
PS_BUFS = int(os.environ.get("MOE_PS_BUFS", "8"))

_prog_cache = {}


def _knot_consts():
    # g_j = exp(32*k_j*t2 + 32*k_j*(1-k_j)) = c_j * q^j,  q = exp((32/7)*t2)
    ks = np.linspace(0.0, 1.0, B_BAS).astype(np.float64)
    cj = np.exp(32.0 * ks * (1.0 - ks))  # c_0 = c_7 = 1
    return ks, cj


def build_program(C, mm_mode, b1_zero):
    """Build + compile the SPMD single-core program for capacity C."""
    import concourse.tile as tile
    from concourse import bacc, mybir

    f32 = mybir.dt.float32
    f32r = mybir.dt.float32r
    bf16 = mybir.dt.bfloat16
    add = mybir.AluOpType.add
    mult = mybir.AluOpType.mult
    Tanh = mybir.ActivationFunctionType.Tanh
    Exp = mybir.ActivationFunctionType.Exp
    Square = mybir.ActivationFunctionType.Square

    mm_dt = f32r
    packed = PACK_MODE == "tile4"
    g_dt = bf16 if packed else f32r

    assert C % P == 0
    tiles = []
    t0 = 0
    while C - t0 >= TNMAX:
        tiles.append((t0, TNMAX))
        t0 += TNMAX
    if C - t0 > 0:
        tiles.append((t0, C - t0))

    _, cj = _knot_consts()
    QS = 32.0 / 7.0  # exp scale

    nc = bacc.Bacc("TRN2", target_bir_lowering=False, debug=False,
                   num_devices=N_CORES)

    xT = nc.dram_tensor("xT", [D_IN, C], mm_dt, kind="ExternalInput").ap()
    w1 = nc.dram_tensor("w1", [D_IN, U_DIM], mm_dt, kind="ExternalInput").ap()
    p5 = nc.dram_tensor("p5", [U_DIM, U_DIM], mm_dt, kind="ExternalInput").ap()
    if packed:
        # auxn[p, vc*7+(j-1), i] = diag content: a_j[vc][p] at i == p%32
        auxn = nc.dram_tensor("auxn", [P, 28, 32], g_dt,
                              kind="ExternalInput").ap()
        auxd = nc.dram_tensor("auxd", [P, 8, 32], g_dt,
                              kind="ExternalInput").ap()
    else:
        auxn = nc.dram_tensor("auxn", [28, P, P], g_dt,
                              kind="ExternalInput").ap()
        auxd = nc.dram_tensor("auxd", [8, P, P], g_dt,
                              kind="ExternalInput").ap()
    a0h = nc.dram_tensor("a0h", [P, 4], f32, kind="ExternalInput").ap()
    onesd = nc.dram_tensor("onesd", [P, TNMAX], g_dt,
                           kind="ExternalInput").ap()
    b1h = nc.dram_tensor("b1h", [P, 4], f32, kind="ExternalInput").ap()
    outT = nc.dram_tensor("outT", [U_DIM, C], f32, kind="ExternalOutput").ap()

    xT_r = xT.rearrange("(kc p) c -> p kc c", p=P)
    if packed:
        auxn_r, auxd_r = auxn, auxd
    else:
        auxn_r = auxn.rearrange("a p q -> p a q")
        auxd_r = auxd.rearrange("a p q -> p a q")
    w1_r = w1.rearrange("(kc p) u -> p kc u", p=P)
    p5_r = p5.rearrange("(uc p) v -> p uc v", p=P)
    outT_r = outT.rearrange("(vc p) c -> p vc c", p=P)

    with tile.TileContext(nc) as tc, ExitStack() as ctx:
        cpool = ctx.enter_context(tc.tile_pool(name="consts", bufs=1))
        xpool = ctx.enter_context(tc.tile_pool(name="x", bufs=X_BUFS))
        pspool = ctx.enter_context(tc.tile_pool(name="ps", bufs=PS_BUFS,
                                                space="PSUM"))
        epool = ctx.enter_context(tc.tile_pool(name="elem", bufs=4))
        swpool = ctx.enter_context(tc.tile_pool(name="sw", bufs=6))
        gpool = ctx.enter_context(tc.tile_pool(name="g", bufs=G_BUFS))
        mpool = ctx.enter_context(tc.tile_pool(name="m", bufs=4))
        opool = ctx.enter_context(tc.tile_pool(name="o", bufs=2))

        # x token tiles: issue ALL loads first so tile 0's data races the
        # (larger) weight loads instead of queueing behind them
        xq = []
        for (t0, TN) in tiles:
            xa = xpool.tile([P, 4, TNMAX], mm_dt, tag="xa", name=f"xa{t0}")
            nc.sync.dma_start(xa[:, :, :TN], xT_r[:, 0:4, t0:t0 + TN])
            xb = xpool.tile([P, 4, TNMAX], mm_dt, tag="xb", name=f"xb{t0}")
            nc.sync.dma_start(xb[:, :, :TN], xT_r[:, 4:8, t0:t0 + TN])
            xq.append((xa, xb))

        # resident weights on the ACT queue (parallel with x on sync)
        w1k = []
        for kc in range(8):
            t = cpool.tile([P, U_DIM], mm_dt, tag=f"w1_{kc}")
            nc.scalar.dma_start(t[:], w1_r[:, kc, :])
            w1k.append(t)
        puc = []
        for uc in range(4):
            t = cpool.tile([P, U_DIM], mm_dt, tag=f"p5_{uc}")
            eng = nc.sync if uc % 2 == 0 else nc.scalar
            eng.dma_start(t[:], p5_r[:, uc, :])
            puc.append(t)
        # small/late-needed constants via the gpsimd SWDGE queue
        if packed:
            auxnsb = cpool.tile([P, 28, 32], g_dt, tag="auxn")
            auxdsb = cpool.tile([P, 8, 32], g_dt, tag="auxd")
        else:
            auxnsb = cpool.tile([P, 28, P], g_dt, tag="auxn")
            auxdsb = cpool.tile([P, 8, P], g_dt, tag="auxd")
        nc.gpsimd.dma_start(auxnsb[:], auxn_r[:])
        nc.gpsimd.dma_start(auxdsb[:], auxd_r[:])
        a0sb = cpool.tile([P, 4], f32, tag="a0h")
        nc.gpsimd.dma_start(a0sb[:], a0h[:])
        ones = cpool.tile([P, TNMAX], g_dt, tag="ones")
        nc.gpsimd.dma_start(ones[:], onesd[:])
        if not b1_zero:
            b1sb = cpool.tile([P, 4], f32, tag="b1h")
            nc.gpsimd.dma_start(b1sb[:], b1h[:])

        for ti, (t0, TN) in enumerate(tiles):
            xa, xb = xq[ti]

            # ---- h = x @ W1 ; sw = 2*swish(h) -----------------------
            sws = []
            for uc in range(4):
                hps = pspool.tile([P, TNMAX], f32, tag="ps", name="hps")
                for kc in range(8):
                    xt = xa if kc < 4 else xb
                    nc.tensor.matmul(
                        hps[:, :TN],
                        lhsT=w1k[kc][:, uc * P:(uc + 1) * P],
                        rhs=xt[:, kc % 4, :TN],
                        start=(kc == 0), stop=(kc == 7),
                    )
                th = epool.tile([P, TNMAX], f32, tag="th")
                if b1_zero:
                    nc.scalar.activation(th[:, :TN], hps[:, :TN], Tanh,
                                         scale=0.5)
                else:
                    nc.scalar.activation(th[:, :TN], hps[:, :TN], Tanh,
                                         scale=0.5, bias=b1sb[:, uc:uc + 1])
                sw = swpool.tile([P, TNMAX], mm_dt, tag="sw")
                if b1_zero:
                    # sw = (th + 1) * h  == 2*swish(h)
                    nc.vector.scalar_tensor_tensor(
                        sw[:, :TN], th[:, :TN], 1.0, hps[:, :TN],
                        op0=add, op1=mult)
                else:
                    y = epool.tile([P, TNMAX], f32, tag="y")
                    nc.vector.tensor_scalar(
                        y[:, :TN], hps[:, :TN], b1sb[:, uc:uc + 1], None,
                        op0=add)
                    nc.vector.scalar_tensor_tensor(
                        sw[:, :TN], th[:, :TN], 1.0, y[:, :TN],
                        op0=add, op1=mult)
                sws.append(sw)

            # ---- z = sw @ (0.5*proj); q powers ----------------------
            pw = []  # pw[vc] = [None, q, q2, ..., q7]
            for vc in range(4):
                zps = pspool.tile([P, TNMAX], f32, tag="ps", name="zps")
                for uc in range(4):
                    nc.tensor.matmul(
                        zps[:, :TN],
                        lhsT=puc[uc][:, vc * P:(vc + 1) * P],
                        rhs=sws[uc][:, :TN],
                        start=(uc == 0), stop=(uc == 3),
                    )
                t2 = epool.tile([P, TNMAX], f32, tag="t2")
                nc.scalar.activation(t2[:, :TN], zps[:, :TN], Tanh, scale=0.5)

                q = [None] * 8
                for j in (1, 2, 3, 4, 5, 6, 7):
                    q[j] = gpool.tile([P, TNMAX], g_dt, tag="g",
                                      name=f"q{j}_{vc}")
                nc.scalar.activation(q[1][:, :TN], t2[:, :TN], Exp, scale=QS)
                # squares on ACT (knob), remaining powers as TT products
                tt_plan = []
                if SQ_ACT >= 1:
                    nc.scalar.activation(q[2][:, :TN], q[1][:, :TN], Square)
                else:
                    tt_plan.append((2, 1, 1))
                if SQ_ACT >= 2:
                    nc.scalar.activation(q[4][:, :TN], q[2][:, :TN], Square)
                else:
                    tt_plan.append((4, 2, 2))
                tt_plan += [(3, 1, 2), (5, 1, 4), (6, 2, 4), (7, 3, 4)]
                for idx, (jo, ja, jb) in enumerate(tt_plan):
                    eng = nc.vector if idx < N_PW_DVE else nc.gpsimd
                    eng.tensor_tensor(q[jo][:, :TN], q[ja][:, :TN],
                                      q[jb][:, :TN], mult)
                pw.append(q)

            # ---- num/den accumulation on PE -------------------------
            outb = opool.tile([P, 4, TNMAX], f32, tag="outb")
            if not packed:
                for vc in range(4):
                    q = pw[vc]
                    nps = pspool.tile([P, TNMAX], f32, tag="ps", name="nps")
                    for j in range(1, 8):
                        nc.tensor.matmul(
                            nps[:, :TN],
                            lhsT=auxnsb[:, vc * 7 + (j - 1), :],
                            rhs=q[j][:, :TN],
                            start=(j == 1), stop=(j == 7))
                    dps = pspool.tile([P, TNMAX], f32, tag="ps", name="dps")
                    for j in range(8):
                        rhs = ones[:, :TN] if j == 0 else q[j][:, :TN]
                        nc.tensor.matmul(
                            dps[:, :TN],
                            lhsT=auxdsb[:, j, :],
                            rhs=rhs,
                            start=(j == 0), stop=(j == 7))
                    r = mpool.tile([P, TNMAX], f32, tag="r", name=f"r{vc}")
                    nc.vector.reciprocal_approx_fast(r[:, :TN], dps[:, :TN])
                    nc.vector.scalar_tensor_tensor(
                        outb[:, vc, :TN], nps[:, :TN], a0sb[:, vc:vc + 1],
                        r[:, :TN], op0=add, op1=mult)
                nc.sync.dma_start(outT_r[:, :, t0:t0 + TN], outb[:, :, :TN])
            else:
                # full-array packed rounds: chain vc at rotation vc.
                nps = [pspool.tile([P, TNMAX], f32, tag="ps",
                                   name=f"nps{vc}") for vc in range(4)]
                dps = [pspool.tile([P, TNMAX], f32, tag="ps",
                                   name=f"dps{vc}") for vc in range(4)]
                for j in range(1, 8):  # num round
                    for vc in range(4):
                        for rr in range(4):
                            cc = (rr + vc) % 4
                            nc.tensor.matmul(
                                nps[vc][cc * 32:(cc + 1) * 32, :TN],
                                lhsT=auxnsb[rr * 32:(rr + 1) * 32,
                                            vc * 7 + (j - 1), :],
                                rhs=pw[vc][j][rr * 32:(rr + 1) * 32, :TN],
                                start=(j == 1), stop=(j == 7),
                                tile_position=(rr * 32, cc * 32))
                for j in range(8):  # den round (j=0 is the ones term)
                    for vc in range(4):
                        for rr in range(4):
                            cc = (rr + vc) % 4
                            rhs = (ones if j == 0 else pw[vc][j])
                            nc.tensor.matmul(
                                dps[vc][cc * 32:(cc + 1) * 32, :TN],
                                lhsT=auxdsb[rr * 32:(rr + 1) * 32, j, :],
                                rhs=rhs[rr * 32:(rr + 1) * 32, :TN],
                                start=(j == 0), stop=(j == 7),
                                tile_position=(rr * 32, cc * 32))
                for vc in range(4):
                    r = mpool.tile([P, TNMAX], f32, tag="r", name=f"r{vc}")
                    nc.vector.reciprocal_approx_fast(r[:, :TN],
                                                     dps[vc][:, :TN])
                    # a0 pre-rotated host-side to match rot(vc)
                    nc.vector.scalar_tensor_tensor(
                        outb[:, vc, :TN], nps[vc][:, :TN],
                        a0sb[:, vc:vc + 1],
                        r[:, :TN], op0=add, op1=mult)
                # out DMA unscramble: SBUF partition block c of vc holds
                # units 32*((c - vc) % 4)
                for vc in range(4):
                    if vc == 0:
                        nc.sync.dma_start(outT_r[:, 0, t0:t0 + TN],
                                          outb[:, 0, :TN])
                    else:
                        nr = (4 - vc) * 32
                        nc.sync.dma_start(
                            outT_r[0:nr, vc, t0:t0 + TN],
                            outb[vc * 32:128, vc, :TN])
                        nc.sync.dma_start(
                            outT_r[nr:128, vc, t0:t0 + TN],
                            outb[0:vc * 32, vc, :TN])

    nc.compile()
    return nc, tiles


def _get_program(C, mm_mode, b1_zero):
    key = (C, mm_mode, b1_zero, PACK_MODE, SQ_ACT, N_PW_DVE, G_BUFS, X_BUFS,
           PS_BUFS)
    if key not in _prog_cache:
        _prog_cache[key] = build_program(C, mm_mode, b1_zero)
    return _prog_cache[key]


def _route_on_host(x, Wg, bg):
    """Expert assignment, bitwise-matching the reference's fp32 CPU math."""
    import jax
    import jax.numpy as jnp

    cpu = jax.devices("cpu")[0]
    with jax.default_device(cpu):
        logits = jnp.asarray(x) @ jnp.asarray(Wg) + jnp.asarray(bg)
        eid = np.asarray(jnp.argmax(logits, axis=-1))
    return eid


def make_in_maps(x, W1, b1, proj, ctrl, scaling, Wg, bg, mm_mode="f32r"):
    import ml_dtypes

    x = np.asarray(x, dtype=np.float32)
    eid = _route_on_host(x, Wg, bg)
    order = np.argsort(eid, kind="stable")
    counts = np.bincount(eid, minlength=E_EXP)
    starts = np.zeros(E_EXP + 1, dtype=np.int64)
    starts[1:] = np.cumsum(counts)
    C = int(max(counts.max(), 1))
    C = ((C + P - 1) // P) * P

    _, cj = _knot_consts()

    cvf = (np.asarray(ctrl, np.float32)
           * np.asarray(scaling, np.float32)[:, None, :])  # [E, B, U]
    proj5 = 0.5 * np.asarray(proj, np.float32)
    b1f = np.asarray(b1, np.float32)
    b1_zero = not np.any(b1f)

    packed = PACK_MODE == "tile4"
    g_np = ml_dtypes.bfloat16 if packed else np.float32
    ar = np.arange(P)

    in_maps = []
    for e in range(E_EXP):
        idx = order[starts[e]:starts[e + 1]]
        xT = np.zeros((D_IN, C), dtype=np.float32)
        if len(idx):
            xT[:, :len(idx)] = x[idx].T
        b1h = np.ascontiguousarray(
            (0.5 * b1f[e]).reshape(4, P).T).astype(np.float32)
        if packed:
            # auxn[p, vc*7+(j-1), i] = a_j[vc][p] if i == p % 32
            auxn = np.zeros((P, 28, 32), dtype=np.float32)
            auxd = np.zeros((P, 8, 32), dtype=np.float32)
            for vc in range(4):
                for j in range(1, 8):
                    w = cvf[e][j, vc * P:(vc + 1) * P] * cj[j]
                    auxn[ar, vc * 7 + (j - 1), ar % 32] = w
            for j in range(8):
                cval = 1.0 if j == 0 else cj[j]
                auxd[ar, j, ar % 32] = cval
        else:
            auxn = np.zeros((28, P, P), dtype=np.float32)
            auxd = np.zeros((8, P, P), dtype=np.float32)
            for vc in range(4):
                for j in range(1, 8):
                    auxn[vc * 7 + (j - 1), ar, ar] = \
                        cvf[e][j, vc * P:(vc + 1) * P] * cj[j]
            for j in range(8):
                auxd[j, ar, ar] = 1.0 if j == 0 else cj[j]
        # a0 = cv_0 per unit, layout [P, vc]; pre-rotated in packed mode
        a0 = np.zeros((P, 4), dtype=np.float32)
        for vc in range(4):
            v = cvf[e][0, vc * P:(vc + 1) * P]
            if packed:
                a0[:, vc] = np.roll(v.reshape(4, 32), vc, axis=0).reshape(P)
            else:
                a0[:, vc] = v
        in_maps.append({
            "xT": xT,
            "w1": np.asarray(W1[e], np.float32),
            "p5": proj5[e],
            "auxn": auxn.astype(g_np),
            "auxd": auxd.astype(g_np),
            "a0h": a0,
            "b1h": b1h,
            "onesd": np.ones((P, TNMAX), dtype=g_np),
        })
    return in_maps, order, starts, counts, C, b1_zero


def kernel(x, W1, b1, proj, ctrl, scaling, Wg, bg):
    from concourse.bass_utils import run_bass_kernel_spmd

    in_maps, order, starts, counts, C, b1_zero = make_in_maps(
        x, W1, b1, proj, ctrl, scaling, Wg, bg)
    nc, _ = _get_program(C, "f32r", b1_zero)

    res = run_bass_kernel_spmd(nc, in_maps, list(range(N_CORES)))

    out = np.empty((N_TOK, U_DIM), dtype=np.float32)
    for e in range(E_EXP):
        cnt = int(counts[e])
        if cnt:
            out[order[starts[e]:starts[e + 1]]] = \
                res.results[e]["outT"][:, :cnt].T
    return out


MM_MODE = "f32r"  # kept for test.py compatibility


# revision 7
# speedup vs baseline: 1.2267x; 1.2267x over previous
"""MoE (top-1 routed) Trainium2 kernel.

Strategy: the reference computes every expert for every token and then
selects one expert per token with a one-hot gate.  Mathematically the
output for token n is expert_out[argmax_e logits[n, e], n], so we compute
the gating on host (bitwise-matching the reference's fp32 `x @ Wg + bg`
on CPU), group tokens by their selected expert, and run expert e's
pipeline for only its own tokens on NeuronCore e (expert-parallel, an
all-reduce-free gather).  This is 8x less device compute than the dense
reference formulation.

Device pipeline per core (C = padded token count, transposed layout with
features on partitions and tokens on the free dim):
    h^T[u, n]  = W1^T x^T          (PE, K=1024 accumulated in PSUM)
    sw         = (tanh(h/2) + 1) * h            # == 2*swish(h)
    z^T[v, n]  = (0.5*proj)^T sw   (PE)         # 0.5 folds the 2 above
    t2         = tanh(z/2)                      # == 2*sigmoid(z) - 1
    q          = exp((32/7) * t2)               # ONE exp per block
      -- the reference's gaussian basis times exp(32*xn^2) is
         g_j = exp(32*k_j*t2 + 32*k_j*(1-k_j)) = c_j * q^j  (k_j = j/7),
         so the basis numerator/denominator are degree-7 polynomials in
         q.  The per-element factor cancels in the normalization (the
         reference's +1e-6 in the denominator is a <=1.2e-6 relative
         perturbation, below fp32 matmul noise).
    powers q^2..q^7 via ACT square + DVE/GPSIMD multiplies (bf16)
    num        = sum_j (cv_j*c_j) q^j   (PE; cv = ctrl * scaling)
    den        = 1 + sum_j c_j q^j      (PE; the 1 via a ones tile)
    out^T[u,n] = (num + cv_0) * reciprocal(den)

tanh/exp/square share one ACT table set ("exp_and_others"), so the
scalar engine never pays the ~2.7us table switch.

num/den accumulate on the PE as diagonal matmuls.  A 128x128 diagonal
weight only occupies the 4 diagonal 32x32 subarrays of the PE, so in
packed mode ("tile4") each accumulation step issues 16 concurrent 32x32
tile_position matmuls: 4 chains (num or den of the 4 unit blocks vc)
at rotations rot=vc, chain vc at positions (32r, 32((r+vc)%4)), each
row-group r consuming that chain's power tile slice [32r:32r+32).
Sub-128 dst partitions are only legal for 16-bit dtypes, hence the
basis runs in bf16 (host-simulated rel err 4.8e-3 vs the 2e-2 budget;
the main matmul path stays f32r).  Chain outputs are partition-block-
rotated by vc; a0 is pre-rotated host-side and the output DMA
unscrambles (2 descriptors per vc).
"""

import os
from contextlib import ExitStack

import numpy as np

N_TOK, D_IN, U_DIM, E_EXP, B_BAS = 8192, 1024, 512, 8, 8
N_CORES = 8
P = 128
TNMAX = 512

PACK_MODE = os.environ.get("MOE_PACK", "plain")  # "plain" | "tile4"
SQ_ACT = int(os.environ.get("MOE_SQ_ACT", "2"))  # q2/q4 via ACT square (0-2)
N_PW_DVE = int(os.environ.get("MOE_PW_DVE", "3"))  # TT powers on DVE (rest GPSIMD)
G_BUFS = int(os.environ.get("MOE_GBUFS", "32"))
X_BUFS = int(os.environ.get("MOE_XBUFS", "2"))
PS_BUFS = int(os.environ.get("MOE_PS_BUFS", "8"))

_prog_cache = {}


def _knot_consts():
    # g_j = exp(32*k_j*t2 + 32*k_j*(1-k_j)) = c_j * q^j,  q = exp((32/7)*t2)
    ks = np.linspace(0.0, 1.0, B_BAS).astype(np.float64)
    cj = np.exp(32.0 * ks * (1.0 - ks))  # c_0 = c_7 = 1
    return ks, cj


def build_program(C, mm_mode, b1_zero):
    """Build + compile the SPMD single-core program for capacity C."""
    import concourse.tile as tile
    from concourse import bacc, mybir

    f32 = mybir.dt.float32
    f32r = mybir.dt.float32r
    bf16 = mybir.dt.bfloat16
    add = mybir.AluOpType.add
    mult = mybir.AluOpType.mult
    Tanh = mybir.ActivationFunctionType.Tanh
    Exp = mybir.ActivationFunctionType.Exp
    Square = mybir.ActivationFunctionType.Square

    mm_dt = f32r
    packed = PACK_MODE == "tile4"
    g_dt = bf16

    assert C % P == 0
    tiles = []
    t0 = 0
    while C - t0 >= TNMAX:
        tiles.append((t0, TNMAX))
        t0 += TNMAX
    if C - t0 > 0:
        tiles.append((t0, C - t0))

    _, cj = _knot_consts()
    QS = 32.0 / 7.0  # exp scale

    nc = bacc.Bacc("TRN2", target_bir_lowering=False, debug=False,
                   num_devices=N_CORES)

    xT = nc.dram_tensor("xT", [D_IN, C], mm_dt, kind="ExternalInput").ap()
    w1 = nc.dram_tensor("w1", [D_IN, U_DIM], mm_dt, kind="ExternalInput").ap()
    p5 = nc.dram_tensor("p5", [U_DIM, U_DIM], mm_dt, kind="ExternalInput").ap()
    if packed:
        # auxn[p, vc*7+(j-1), i] = diag content: a_j[vc][p] at i == p%32
        auxn = nc.dram_tensor("auxn", [P, 28, 32], g_dt,
                              kind="ExternalInput").ap()
        auxd = nc.dram_tensor("auxd", [P, 8, 32], g_dt,
                              kind="ExternalInput").ap()
    else:
        auxn = nc.dram_tensor("auxn", [28, P, P], g_dt,
                              kind="ExternalInput").ap()
        auxd = nc.dram_tensor("auxd", [8, P, P], g_dt,
                              kind="ExternalInput").ap()
    a0h = nc.dram_tensor("a0h", [P, 4], f32, kind="ExternalInput").ap()
    onesd = nc.dram_tensor("onesd", [P, TNMAX], g_dt,
                           kind="ExternalInput").ap()
    b1h = nc.dram_tensor("b1h", [P, 4], f32, kind="ExternalInput").ap()
    outT = nc.dram_tensor("outT", [U_DIM, C], f32, kind="ExternalOutput").ap()

    xT_r = xT.rearrange("(kc p) c -> p kc c", p=P)
    if packed:
        auxn_r, auxd_r = auxn, auxd
    else:
        auxn_r = auxn.rearrange("a p q -> p a q")
        auxd_r = auxd.rearrange("a p q -> p a q")
    w1_r = w1.rearrange("(kc p) u -> p kc u", p=P)
    p5_r = p5.rearrange("(uc p) v -> p uc v", p=P)
    outT_r = outT.rearrange("(vc p) c -> p vc c", p=P)

    with tile.TileContext(nc) as tc, ExitStack() as ctx:
        cpool = ctx.enter_context(tc.tile_pool(name="consts", bufs=1))
        xpool = ctx.enter_context(tc.tile_pool(name="x", bufs=X_BUFS))
        pspool = ctx.enter_context(tc.tile_pool(name="ps", bufs=PS_BUFS,
                                                space="PSUM"))
        epool = ctx.enter_context(tc.tile_pool(name="elem", bufs=4))
        swpool = ctx.enter_context(tc.tile_pool(name="sw", bufs=6))
        gpool = ctx.enter_context(tc.tile_pool(name="g", bufs=G_BUFS))
        mpool = ctx.enter_context(tc.tile_pool(name="m", bufs=4))
        opool = ctx.enter_context(tc.tile_pool(name="o", bufs=2))

        # x token tiles: issue ALL loads first so tile 0's data races the
        # (larger) weight loads instead of queueing behind them
        xq = []
        for (t0, TN) in tiles:
            xa = xpool.tile([P, 4, TNMAX], mm_dt, tag="xa", name=f"xa{t0}")
            nc.sync.dma_start(xa[:, :, :TN], xT_r[:, 0:4, t0:t0 + TN])
            xb = xpool.tile([P, 4, TNMAX], mm_dt, tag="xb", name=f"xb{t0}")
            nc.sync.dma_start(xb[:, :, :TN], xT_r[:, 4:8, t0:t0 + TN])
            xq.append((xa, xb))

        # resident weights on the ACT queue (parallel with x on sync)
        w1k = []
        for kc in range(8):
            t = cpool.tile([P, U_DIM], mm_dt, tag=f"w1_{kc}")
            nc.scalar.dma_start(t[:], w1_r[:, kc, :])
            w1k.append(t)
        puc = []
        for uc in range(4):
            t = cpool.tile([P, U_DIM], mm_dt, tag=f"p5_{uc}")
            eng = nc.sync if uc % 2 == 0 else nc.scalar
            eng.dma_start(t[:], p5_r[:, uc, :])
            puc.append(t)
        # small/late-needed constants via the gpsimd SWDGE queue
        if packed:
            auxnsb = cpool.tile([P, 28, 32], g_dt, tag="auxn")
            auxdsb = cpool.tile([P, 8, 32], g_dt, tag="auxd")
        else:
            auxnsb = cpool.tile([P, 28, P], g_dt, tag="auxn")
            auxdsb = cpool.tile([P, 8, P], g_dt, tag="auxd")
        nc.gpsimd.dma_start(auxnsb[:], auxn_r[:])
        nc.gpsimd.dma_start(auxdsb[:], auxd_r[:])
        a0sb = cpool.tile([P, 4], f32, tag="a0h")
        nc.gpsimd.dma_start(a0sb[:], a0h[:])
        ones = cpool.tile([P, TNMAX], g_dt, tag="ones")
        nc.gpsimd.dma_start(ones[:], onesd[:])
        if not b1_zero:
            b1sb = cpool.tile([P, 4], f32, tag="b1h")
            nc.gpsimd.dma_start(b1sb[:], b1h[:])

        for ti, (t0, TN) in enumerate(tiles):
            xa, xb = xq[ti]

            # ---- h = x @ W1 ; sw = 2*swish(h) -----------------------
            sws = []
            for uc in range(4):
                hps = pspool.tile([P, TNMAX], f32, tag="ps", name="hps")
                for kc in range(8):
                    xt = xa if kc < 4 else xb
                    nc.tensor.matmul(
                        hps[:, :TN],
                        lhsT=w1k[kc][:, uc * P:(uc + 1) * P],
                        rhs=xt[:, kc % 4, :TN],
                        start=(kc == 0), stop=(kc == 7),
                    )
                th = epool.tile([P, TNMAX], f32, tag="th")
                if b1_zero:
                    nc.scalar.activation(th[:, :TN], hps[:, :TN], Tanh,
                                         scale=0.5)
                else:
                    nc.scalar.activation(th[:, :TN], hps[:, :TN], Tanh,
                                         scale=0.5, bias=b1sb[:, uc:uc + 1])
                sw = swpool.tile([P, TNMAX], mm_dt, tag="sw")
                if b1_zero:
                    # sw = (th + 1) * h  == 2*swish(h)
                    nc.vector.scalar_tensor_tensor(
                        sw[:, :TN], th[:, :TN], 1.0, hps[:, :TN],
                        op0=add, op1=mult)
                else:
                    y = epool.tile([P, TNMAX], f32, tag="y")
                    nc.vector.tensor_scalar(
                        y[:, :TN], hps[:, :TN], b1sb[:, uc:uc + 1], None,
                        op0=add)
                    nc.vector.scalar_tensor_tensor(
                        sw[:, :TN], th[:, :TN], 1.0, y[:, :TN],
                        op0=add, op1=mult)
                sws.append(sw)

            # ---- z = sw @ (0.5*proj); q powers ----------------------
            pw = []  # pw[vc] = [None, q, q2, ..., q7]
            for vc in range(4):
                zps = pspool.tile([P, TNMAX], f32, tag="ps", name="zps")
                for uc in range(4):
                    nc.tensor.matmul(
                        zps[:, :TN],
                        lhsT=puc[uc][:, vc * P:(vc + 1) * P],
                        rhs=sws[uc][:, :TN],
                        start=(uc == 0), stop=(uc == 3),
                    )
                t2 = epool.tile([P, TNMAX], f32, tag="t2")
                nc.scalar.activation(t2[:, :TN], zps[:, :TN], Tanh, scale=0.5)

                q = [None] * 8
                for j in (1, 2, 3, 4, 5, 6, 7):
                    q[j] = gpool.tile([P, TNMAX], g_dt, tag="g",
                                      name=f"q{j}_{vc}")
                nc.scalar.activation(q[1][:, :TN], t2[:, :TN], Exp, scale=QS)
                # squares on ACT (knob), remaining powers as TT products
                tt_plan = []
                if SQ_ACT >= 1:
                    nc.scalar.activation(q[2][:, :TN], q[1][:, :TN], Square)
                else:
                    tt_plan.append((2, 1, 1))
                if SQ_ACT >= 2:
                    nc.scalar.activation(q[4][:, :TN], q[2][:, :TN], Square)
                else:
                    tt_plan.append((4, 2, 2))
                tt_plan += [(3, 1, 2), (5, 1, 4), (6, 2, 4), (7, 3, 4)]
                for idx, (jo, ja, jb) in enumerate(tt_plan):
                    eng = nc.vector if idx < N_PW_DVE else nc.gpsimd
                    eng.tensor_tensor(q[jo][:, :TN], q[ja][:, :TN],
                                      q[jb][:, :TN], mult)
                pw.append(q)

            # ---- num/den accumulation on PE -------------------------
            outb = opool.tile([P, 4, TNMAX], f32, tag="outb")
            if not packed:
                for vc in range(4):
                    q = pw[vc]
                    nps = pspool.tile([P, TNMAX], f32, tag="ps", name="nps")
                    dps = pspool.tile([P, TNMAX], f32, tag="ps", name="dps")
                    # interleave num/den so q_j frees right after its pair
                    for j in range(1, 8):
                        nc.tensor.matmul(
                            nps[:, :TN],
                            lhsT=auxnsb[:, vc * 7 + (j - 1), :],
                            rhs=q[j][:, :TN],
                            start=(j == 1), stop=(j == 7))
                        nc.tensor.matmul(
                            dps[:, :TN],
                            lhsT=auxdsb[:, j, :],
                            rhs=q[j][:, :TN],
                            start=(j == 1), stop=False)
                    nc.tensor.matmul(
                        dps[:, :TN],
                        lhsT=auxdsb[:, 0, :],
                        rhs=ones[:, :TN],
                        start=False, stop=True)
                    r = mpool.tile([P, TNMAX], f32, tag="r", name=f"r{vc}")
                    nc.vector.reciprocal_approx_fast(r[:, :TN], dps[:, :TN])
                    nc.vector.scalar_tensor_tensor(
                        outb[:, vc, :TN], nps[:, :TN], a0sb[:, vc:vc + 1],
                        r[:, :TN], op0=add, op1=mult)
                nc.sync.dma_start(outT_r[:, :, t0:t0 + TN], outb[:, :, :TN])
            else:
                # full-array packed rounds: chain vc at rotation vc.
                nps = [pspool.tile([P, TNMAX], f32, tag="ps",
                                   name=f"nps{vc}") for vc in range(4)]
                dps = [pspool.tile([P, TNMAX], f32, tag="ps",
                                   name=f"dps{vc}") for vc in range(4)]
                for j in range(1, 8):  # num round
                    for vc in range(4):
                        for rr in range(4):
                            cc = (rr + vc) % 4
                            nc.tensor.matmul(
                                nps[vc][cc * 32:(cc + 1) * 32, :TN],
                                lhsT=auxnsb[rr * 32:(rr + 1) * 32,
                                            vc * 7 + (j - 1), :],
                                rhs=pw[vc][j][rr * 32:(rr + 1) * 32, :TN],
                                start=(j == 1), stop=(j == 7),
                                tile_position=(rr * 32, cc * 32))
                for j in range(8):  # den round (j=0 is the ones term)
                    for vc in range(4):
                        for rr in range(4):
                            cc = (rr + vc) % 4
                            rhs = (ones if j == 0 else pw[vc][j])
                            nc.tensor.matmul(
                                dps[vc][cc * 32:(cc + 1) * 32, :TN],
                                lhsT=auxdsb[rr * 32:(rr + 1) * 32, j, :],
                                rhs=rhs[rr * 32:(rr + 1) * 32, :TN],
                                start=(j == 0), stop=(j == 7),
                                tile_position=(rr * 32, cc * 32))
                for vc in range(4):
                    r = mpool.tile([P, TNMAX], f32, tag="r", name=f"r{vc}")
                    nc.vector.reciprocal_approx_fast(r[:, :TN],
                                                     dps[vc][:, :TN])
                    # a0 pre-rotated host-side to match rot(vc)
                    nc.vector.scalar_tensor_tensor(
                        outb[:, vc, :TN], nps[vc][:, :TN],
                        a0sb[:, vc:vc + 1],
                        r[:, :TN], op0=add, op1=mult)
                # out DMA unscramble: SBUF partition block c of vc holds
                # units 32*((c - vc) % 4)
                for vc in range(4):
                    if vc == 0:
                        nc.sync.dma_start(outT_r[:, 0, t0:t0 + TN],
                                          outb[:, 0, :TN])
                    else:
                        nr = (4 - vc) * 32
                        nc.sync.dma_start(
                            outT_r[0:nr, vc, t0:t0 + TN],
                            outb[vc * 32:128, vc, :TN])
                        nc.sync.dma_start(
                            outT_r[nr:128, vc, t0:t0 + TN],
                            outb[0:vc * 32, vc, :TN])

    nc.compile()
    return nc, tiles


def _get_program(C, mm_mode, b1_zero):
    key = (C, mm_mode, b1_zero, PACK_MODE, SQ_ACT, N_PW_DVE, G_BUFS, X_BUFS,
           PS_BUFS)
    if key not in _prog_cache:
        _prog_cache[key] = build_program(C, mm_mode, b1_zero)
    return _prog_cache[key]


def _route_on_host(x, Wg, bg):
    """Expert assignment, bitwise-matching the reference's fp32 CPU math."""
    import jax
    import jax.numpy as jnp

    cpu = jax.devices("cpu")[0]
    with jax.default_device(cpu):
        logits = jnp.asarray(x) @ jnp.asarray(Wg) + jnp.asarray(bg)
        eid = np.asarray(jnp.argmax(logits, axis=-1))
    return eid


def make_in_maps(x, W1, b1, proj, ctrl, scaling, Wg, bg, mm_mode="f32r"):
    import ml_dtypes

    x = np.asarray(x, dtype=np.float32)
    eid = _route_on_host(x, Wg, bg)
    order = np.argsort(eid, kind="stable")
    counts = np.bincount(eid, minlength=E_EXP)
    starts = np.zeros(E_EXP + 1, dtype=np.int64)
    starts[1:] = np.cumsum(counts)
    C = int(max(counts.max(), 1))
    C = ((C + P - 1) // P) * P

    _, cj = _knot_consts()

    cvf = (np.asarray(ctrl, np.float32)
           * np.asarray(scaling, np.float32)[:, None, :])  # [E, B, U]
    proj5 = 0.5 * np.asarray(proj, np.float32)
    b1f = np.asarray(b1, np.float32)
    b1_zero = not np.any(b1f)

    packed = PACK_MODE == "tile4"
    g_np = ml_dtypes.bfloat16
    ar = np.arange(P)

    in_maps = []
    for e in range(E_EXP):
        idx = order[starts[e]:starts[e + 1]]
        xT = np.zeros((D_IN, C), dtype=np.float32)
        if len(idx):
            xT[:, :len(idx)] = x[idx].T
        b1h = np.ascontiguousarray(
            (0.5 * b1f[e]).reshape(4, P).T).astype(np.float32)
        if packed:
            # auxn[p, vc*7+(j-1), i] = a_j[vc][p] if i == p % 32
            auxn = np.zeros((P, 28, 32), dtype=np.float32)
            auxd = np.zeros((P, 8, 32), dtype=np.float32)
            for vc in range(4):
                for j in range(1, 8):
                    w = cvf[e][j, vc * P:(vc + 1) * P] * cj[j]
                    auxn[ar, vc * 7 + (j - 1), ar % 32] = w
            for j in range(8):
                cval = 1.0 if j == 0 else cj[j]
                auxd[ar, j, ar % 32] = cval
        else:
            auxn = np.zeros((28, P, P), dtype=np.float32)
            auxd = np.zeros((8, P, P), dtype=np.float32)
            for vc in range(4):
                for j in range(1, 8):
                    auxn[vc * 7 + (j - 1), ar, ar] = \
                        cvf[e][j, vc * P:(vc + 1) * P] * cj[j]
            for j in range(8):
                auxd[j, ar, ar] = 1.0 if j == 0 else cj[j]
        # a0 = cv_0 per unit, layout [P, vc]; pre-rotated in packed mode
        a0 = np.zeros((P, 4), dtype=np.float32)
        for vc in range(4):
            v = cvf[e][0, vc * P:(vc + 1) * P]
            if packed:
                a0[:, vc] = np.roll(v.reshape(4, 32), vc, axis=0).reshape(P)
            else:
                a0[:, vc] = v
        in_maps.append({
            "xT": xT,
            "w1": np.asarray(W1[e], np.float32),
            "p5": proj5[e],
            "auxn": auxn.astype(g_np),
            "auxd": auxd.astype(g_np),
            "a0h": a0,
            "b1h": b1h,
            "onesd": np.ones((P, TNMAX), dtype=g_np),
        })
    return in_maps, order, starts, counts, C, b1_zero


def kernel(x, W1, b1, proj, ctrl, scaling, Wg, bg):
    from concourse.bass_utils import run_bass_kernel_spmd

    in_maps, order, starts, counts, C, b1_zero = make_in_maps(
        x, W1, b1, proj, ctrl, scaling, Wg, bg)
    nc, _ = _get_program(C, "f32r", b1_zero)

    res = run_bass_kernel_spmd(nc, in_maps, list(range(N_CORES)))

    out = np.empty((N_TOK, U_DIM), dtype=np.float32)
    for e in range(E_EXP):
        cnt = int(counts[e])
        if cnt:
            out[order[starts[e]:starts[e + 1]]] = \
                res.results[e]["outT"][:, :cnt].T
    return out


MM_MODE = "f32r"  # kept for test.py compatibility


# revision 8
# speedup vs baseline: 1.3910x; 1.1339x over previous
"""MoE (top-1 routed) Trainium2 kernel.

Strategy: the reference computes every expert for every token and then
selects one expert per token with a one-hot gate.  Mathematically the
output for token n is expert_out[argmax_e logits[n, e], n], so we compute
the gating on host (bitwise-matching the reference's fp32 `x @ Wg + bg`
on CPU), group tokens by their selected expert, and run expert e's
pipeline for only its own tokens on NeuronCore e (expert-parallel, an
all-reduce-free gather).  This is 8x less device compute than the dense
reference formulation.

Device pipeline per core (C = padded token count, transposed layout with
features on partitions and tokens on the free dim):
    h^T[u, n]  = W1^T x^T          (PE, K=1024 accumulated in PSUM)
    sw         = (tanh(h/2) + 1) * h            # == 2*swish(h)
    z^T[v, n]  = (0.5*proj)^T sw   (PE)         # 0.5 folds the 2 above
    t2         = tanh(z/2)                      # == 2*sigmoid(z) - 1
    q          = exp((32/7) * t2)               # ONE exp per block
      -- the reference's gaussian basis times exp(32*xn^2) is
         g_j = exp(32*k_j*t2 + 32*k_j*(1-k_j)) = c_j * q^j  (k_j = j/7),
         so the basis numerator/denominator are degree-7 polynomials in
         q.  The per-element factor cancels in the normalization (the
         reference's +1e-6 in the denominator is a <=1.2e-6 relative
         perturbation, below fp32 matmul noise).
    powers q^2..q^7 via ACT square + DVE/GPSIMD multiplies (bf16)
    num        = sum_j (cv_j*c_j) q^j   (PE; cv = ctrl * scaling)
    den        = 1 + sum_j c_j q^j      (PE; the 1 via a ones tile)
    out^T[u,n] = (num + cv_0) * reciprocal(den)

tanh/exp/square share one ACT table set ("exp_and_others"), so the
scalar engine never pays the ~2.7us table switch.

num/den accumulate on the PE as diagonal matmuls.  A 128x128 diagonal
weight only occupies the 4 diagonal 32x32 subarrays of the PE, so in
packed mode ("tile4") each accumulation step issues 16 concurrent 32x32
tile_position matmuls: 4 chains (num or den of the 4 unit blocks vc)
at rotations rot=vc, chain vc at positions (32r, 32((r+vc)%4)), each
row-group r consuming that chain's power tile slice [32r:32r+32).
Sub-128 dst partitions are only legal for 16-bit dtypes, hence the
basis runs in bf16 (host-simulated rel err 4.8e-3 vs the 2e-2 budget;
the main matmul path stays f32r).  Chain outputs are partition-block-
rotated by vc; a0 is pre-rotated host-side and the output DMA
unscrambles (2 descriptors per vc).
"""

import os
from contextlib import ExitStack

import numpy as np

N_TOK, D_IN, U_DIM, E_EXP, B_BAS = 8192, 1024, 512, 8, 8
N_CORES = 8
P = 128
TNMAX = 512

PACK_MODE = os.environ.get("MOE_PACK", "plain")  # "plain" | "tile4"
SQ_ACT = int(os.environ.get("MOE_SQ_ACT", "2"))  # q2/q4 via ACT square (0-2)
N_PW_DVE = int(os.environ.get("MOE_PW_DVE", "3"))  # TT powers on DVE (rest GPSIMD)
G_BUFS = int(os.environ.get("MOE_GBUFS", "56"))
X_BUFS = int(os.environ.get("MOE_XBUFS", "3"))
PS_BUFS = int(os.environ.get("MOE_PS_BUFS", "8"))

_prog_cache = {}


def _knot_consts():
    # g_j = exp(32*k_j*t2 + 32*k_j*(1-k_j)) = c_j * q^j,  q = exp((32/7)*t2)
    ks = np.linspace(0.0, 1.0, B_BAS).astype(np.float64)
    cj = np.exp(32.0 * ks * (1.0 - ks))  # c_0 = c_7 = 1
    return ks, cj


def build_program(C, mm_mode, b1_zero):
    """Build + compile the SPMD single-core program for capacity C."""
    import concourse.tile as tile
    from concourse import bacc, mybir

    f32 = mybir.dt.float32
    f32r = mybir.dt.float32r
    bf16 = mybir.dt.bfloat16
    add = mybir.AluOpType.add
    mult = mybir.AluOpType.mult
    Tanh = mybir.ActivationFunctionType.Tanh
    Exp = mybir.ActivationFunctionType.Exp
    Square = mybir.ActivationFunctionType.Square

    mm_dt = f32r
    packed = PACK_MODE == "tile4"
    g_dt = bf16

    assert C % P == 0
    tiles = []
    t0 = 0
    while C - t0 >= TNMAX:
        tiles.append((t0, TNMAX))
        t0 += TNMAX
    if C - t0 > 0:
        tiles.append((t0, C - t0))

    _, cj = _knot_consts()
    QS = 32.0 / 7.0  # exp scale

    nc = bacc.Bacc("TRN2", target_bir_lowering=False, debug=False,
                   num_devices=N_CORES)

    xT = nc.dram_tensor("xT", [D_IN, C], mm_dt, kind="ExternalInput").ap()
    w1 = nc.dram_tensor("w1", [D_IN, U_DIM], mm_dt, kind="ExternalInput").ap()
    p5 = nc.dram_tensor("p5", [U_DIM, U_DIM], mm_dt, kind="ExternalInput").ap()
    if packed:
        # auxn[p, vc*7+(j-1), i] = diag content: a_j[vc][p] at i == p%32
        auxn = nc.dram_tensor("auxn", [P, 28, 32], g_dt,
                              kind="ExternalInput").ap()
        auxd = nc.dram_tensor("auxd", [P, 8, 32], g_dt,
                              kind="ExternalInput").ap()
    else:
        auxn = nc.dram_tensor("auxn", [28, P, P], g_dt,
                              kind="ExternalInput").ap()
        auxd = nc.dram_tensor("auxd", [8, P, P], g_dt,
                              kind="ExternalInput").ap()
    a0h = nc.dram_tensor("a0h", [P, 4], f32, kind="ExternalInput").ap()
    onesd = nc.dram_tensor("onesd", [P, TNMAX], g_dt,
                           kind="ExternalInput").ap()
    b1h = nc.dram_tensor("b1h", [P, 4], f32, kind="ExternalInput").ap()
    outT = nc.dram_tensor("outT", [U_DIM, C], f32, kind="ExternalOutput").ap()

    xT_r = xT.rearrange("(kc p) c -> p kc c", p=P)
    if packed:
        auxn_r, auxd_r = auxn, auxd
    else:
        auxn_r = auxn.rearrange("a p q -> p a q")
        auxd_r = auxd.rearrange("a p q -> p a q")
    w1_r = w1.rearrange("(kc p) u -> p kc u", p=P)
    p5_r = p5.rearrange("(uc p) v -> p uc v", p=P)
    outT_r = outT.rearrange("(vc p) c -> p vc c", p=P)

    with tile.TileContext(nc) as tc, ExitStack() as ctx:
        cpool = ctx.enter_context(tc.tile_pool(name="consts", bufs=1))
        xpool = ctx.enter_context(tc.tile_pool(name="x", bufs=X_BUFS))
        pspool = ctx.enter_context(tc.tile_pool(name="ps", bufs=PS_BUFS,
                                                space="PSUM"))
        epool = ctx.enter_context(tc.tile_pool(name="elem", bufs=4))
        swpool = ctx.enter_context(tc.tile_pool(name="sw", bufs=6))
        gpool = ctx.enter_context(tc.tile_pool(name="g", bufs=G_BUFS))
        mpool = ctx.enter_context(tc.tile_pool(name="m", bufs=4))
        opool = ctx.enter_context(tc.tile_pool(name="o", bufs=2))

        # x token tiles: issue ALL loads first so tile 0's data races the
        # (larger) weight loads instead of queueing behind them
        xq = []
        for (t0, TN) in tiles:
            xa = xpool.tile([P, 4, TNMAX], mm_dt, tag="xa", name=f"xa{t0}")
            nc.sync.dma_start(xa[:, :, :TN], xT_r[:, 0:4, t0:t0 + TN])
            xb = xpool.tile([P, 4, TNMAX], mm_dt, tag="xb", name=f"xb{t0}")
            nc.sync.dma_start(xb[:, :, :TN], xT_r[:, 4:8, t0:t0 + TN])
            xq.append((xa, xb))

        # resident weights on the ACT queue (parallel with x on sync)
        w1k = []
        for kc in range(8):
            t = cpool.tile([P, U_DIM], mm_dt, tag=f"w1_{kc}")
            nc.scalar.dma_start(t[:], w1_r[:, kc, :])
            w1k.append(t)
        puc = []
        for uc in range(4):
            t = cpool.tile([P, U_DIM], mm_dt, tag=f"p5_{uc}")
            eng = nc.sync if uc % 2 == 0 else nc.scalar
            eng.dma_start(t[:], p5_r[:, uc, :])
            puc.append(t)
        # small/late-needed constants via the gpsimd SWDGE queue
        if packed:
            auxnsb = cpool.tile([P, 28, 32], g_dt, tag="auxn")
            auxdsb = cpool.tile([P, 8, 32], g_dt, tag="auxd")
        else:
            auxnsb = cpool.tile([P, 28, P], g_dt, tag="auxn")
            auxdsb = cpool.tile([P, 8, P], g_dt, tag="auxd")
        nc.gpsimd.dma_start(auxnsb[:], auxn_r[:])
        nc.gpsimd.dma_start(auxdsb[:], auxd_r[:])
        a0sb = cpool.tile([P, 4], f32, tag="a0h")
        nc.gpsimd.dma_start(a0sb[:], a0h[:])
        ones = cpool.tile([P, TNMAX], g_dt, tag="ones")
        nc.gpsimd.dma_start(ones[:], onesd[:])
        if not b1_zero:
            b1sb = cpool.tile([P, 4], f32, tag="b1h")
            nc.gpsimd.dma_start(b1sb[:], b1h[:])

        def stage_a(ti):
            """h/sw/z/tanh/powers for tile ti (PE: 48 MMs, ACT/DVE/GPS
            elementwise).  Returns the bf16 power tiles."""
            t0, TN = tiles[ti]
            xa, xb = xq[ti]
            sws = []
            for uc in range(4):
                hps = pspool.tile([P, TNMAX], f32, tag="ps", name="hps")
                for kc in range(8):
                    xt = xa if kc < 4 else xb
                    nc.tensor.matmul(
                        hps[:, :TN],
                        lhsT=w1k[kc][:, uc * P:(uc + 1) * P],
                        rhs=xt[:, kc % 4, :TN],
                        start=(kc == 0), stop=(kc == 7),
                    )
                th = epool.tile([P, TNMAX], f32, tag="th")
                if b1_zero:
                    nc.scalar.activation(th[:, :TN], hps[:, :TN], Tanh,
                                         scale=0.5)
                else:
                    nc.scalar.activation(th[:, :TN], hps[:, :TN], Tanh,
                                         scale=0.5, bias=b1sb[:, uc:uc + 1])
                sw = swpool.tile([P, TNMAX], mm_dt, tag="sw")
                if b1_zero:
                    # sw = (th + 1) * h  == 2*swish(h)
                    nc.vector.scalar_tensor_tensor(
                        sw[:, :TN], th[:, :TN], 1.0, hps[:, :TN],
                        op0=add, op1=mult)
                else:
                    y = epool.tile([P, TNMAX], f32, tag="y")
                    nc.vector.tensor_scalar(
                        y[:, :TN], hps[:, :TN], b1sb[:, uc:uc + 1], None,
                        op0=add)
                    nc.vector.scalar_tensor_tensor(
                        sw[:, :TN], th[:, :TN], 1.0, y[:, :TN],
                        op0=add, op1=mult)
                sws.append(sw)

            pw = []  # pw[vc] = [None, q, q2, ..., q7]
            for vc in range(4):
                zps = pspool.tile([P, TNMAX], f32, tag="ps", name="zps")
                for uc in range(4):
                    nc.tensor.matmul(
                        zps[:, :TN],
                        lhsT=puc[uc][:, vc * P:(vc + 1) * P],
                        rhs=sws[uc][:, :TN],
                        start=(uc == 0), stop=(uc == 3),
                    )
                t2 = epool.tile([P, TNMAX], f32, tag="t2")
                nc.scalar.activation(t2[:, :TN], zps[:, :TN], Tanh, scale=0.5)

                q = [None] * 8
                for j in (1, 2, 3, 4, 5, 6, 7):
                    q[j] = gpool.tile([P, TNMAX], g_dt, tag="g",
                                      name=f"q{j}_{vc}")
                nc.scalar.activation(q[1][:, :TN], t2[:, :TN], Exp, scale=QS)
                # squares on ACT (knob), remaining powers as TT products
                tt_plan = []
                if SQ_ACT >= 1:
                    nc.scalar.activation(q[2][:, :TN], q[1][:, :TN], Square)
                else:
                    tt_plan.append((2, 1, 1))
                if SQ_ACT >= 2:
                    nc.scalar.activation(q[4][:, :TN], q[2][:, :TN], Square)
                else:
                    tt_plan.append((4, 2, 2))
                tt_plan += [(3, 1, 2), (5, 1, 4), (6, 2, 4), (7, 3, 4)]
                for idx, (jo, ja, jb) in enumerate(tt_plan):
                    eng = nc.vector if idx < N_PW_DVE else nc.gpsimd
                    eng.tensor_tensor(q[jo][:, :TN], q[ja][:, :TN],
                                      q[jb][:, :TN], mult)
                pw.append(q)
            return pw

        def stage_b(ti, pw):
            """num/den diag-matmul chains + recip/final + out DMA."""
            t0, TN = tiles[ti]
            outb = opool.tile([P, 4, TNMAX], f32, tag="outb")
            for vc in range(4):
                q = pw[vc]
                nps = pspool.tile([P, TNMAX], f32, tag="ps", name="nps")
                dps = pspool.tile([P, TNMAX], f32, tag="ps", name="dps")
                # interleave num/den so q_j frees right after its pair
                for j in range(1, 8):
                    nc.tensor.matmul(
                        nps[:, :TN],
                        lhsT=auxnsb[:, vc * 7 + (j - 1), :],
                        rhs=q[j][:, :TN],
                        start=(j == 1), stop=(j == 7))
                    nc.tensor.matmul(
                        dps[:, :TN],
                        lhsT=auxdsb[:, j, :],
                        rhs=q[j][:, :TN],
                        start=(j == 1), stop=False)
                nc.tensor.matmul(
                    dps[:, :TN],
                    lhsT=auxdsb[:, 0, :],
                    rhs=ones[:, :TN],
                    start=False, stop=True)
                r = mpool.tile([P, TNMAX], f32, tag="r", name=f"r{vc}")
                nc.vector.reciprocal_approx_fast(r[:, :TN], dps[:, :TN])
                nc.vector.scalar_tensor_tensor(
                    outb[:, vc, :TN], nps[:, :TN], a0sb[:, vc:vc + 1],
                    r[:, :TN], op0=add, op1=mult)
            nc.sync.dma_start(outT_r[:, :, t0:t0 + TN], outb[:, :, :TN])

        # software pipeline: tile i+1's h/z matmuls sit between tile i's
        # z and tile i's num/den in the PE queue, covering the ACT/DVE
        # power-chain latency so the PE never idles (keeps HAM at 8/8)
        prev = None
        for ti in range(len(tiles)):
            cur = stage_a(ti)
            if prev is not None:
                stage_b(ti - 1, prev)
            prev = cur
        stage_b(len(tiles) - 1, prev)

    nc.compile()
    return nc, tiles


def _get_program(C, mm_mode, b1_zero):
    key = (C, mm_mode, b1_zero, PACK_MODE, SQ_ACT, N_PW_DVE, G_BUFS, X_BUFS,
           PS_BUFS)
    if key not in _prog_cache:
        _prog_cache[key] = build_program(C, mm_mode, b1_zero)
    return _prog_cache[key]


def _route_on_host(x, Wg, bg):
    """Expert assignment, bitwise-matching the reference's fp32 CPU math."""
    import jax
    import jax.numpy as jnp

    cpu = jax.devices("cpu")[0]
    with jax.default_device(cpu):
        logits = jnp.asarray(x) @ jnp.asarray(Wg) + jnp.asarray(bg)
        eid = np.asarray(jnp.argmax(logits, axis=-1))
    return eid


def make_in_maps(x, W1, b1, proj, ctrl, scaling, Wg, bg, mm_mode="f32r"):
    import ml_dtypes

    x = np.asarray(x, dtype=np.float32)
    eid = _route_on_host(x, Wg, bg)
    order = np.argsort(eid, kind="stable")
    counts = np.bincount(eid, minlength=E_EXP)
    starts = np.zeros(E_EXP + 1, dtype=np.int64)
    starts[1:] = np.cumsum(counts)
    C = int(max(counts.max(), 1))
    C = ((C + P - 1) // P) * P

    _, cj = _knot_consts()

    cvf = (np.asarray(ctrl, np.float32)
           * np.asarray(scaling, np.float32)[:, None, :])  # [E, B, U]
    proj5 = 0.5 * np.asarray(proj, np.float32)
    b1f = np.asarray(b1, np.float32)
    b1_zero = not np.any(b1f)

    packed = PACK_MODE == "tile4"
    g_np = ml_dtypes.bfloat16
    ar = np.arange(P)

    in_maps = []
    for e in range(E_EXP):
        idx = order[starts[e]:starts[e + 1]]
        xT = np.zeros((D_IN, C), dtype=np.float32)
        if len(idx):
            xT[:, :len(idx)] = x[idx].T
        b1h = np.ascontiguousarray(
            (0.5 * b1f[e]).reshape(4, P).T).astype(np.float32)
        if packed:
            # auxn[p, vc*7+(j-1), i] = a_j[vc][p] if i == p % 32
            auxn = np.zeros((P, 28, 32), dtype=np.float32)
            auxd = np.zeros((P, 8, 32), dtype=np.float32)
            for vc in range(4):
                for j in range(1, 8):
                    w = cvf[e][j, vc * P:(vc + 1) * P] * cj[j]
                    auxn[ar, vc * 7 + (j - 1), ar % 32] = w
            for j in range(8):
                cval = 1.0 if j == 0 else cj[j]
                auxd[ar, j, ar % 32] = cval
        else:
            auxn = np.zeros((28, P, P), dtype=np.float32)
            auxd = np.zeros((8, P, P), dtype=np.float32)
            for vc in range(4):
                for j in range(1, 8):
                    auxn[vc * 7 + (j - 1), ar, ar] = \
                        cvf[e][j, vc * P:(vc + 1) * P] * cj[j]
            for j in range(8):
                auxd[j, ar, ar] = 1.0 if j == 0 else cj[j]
        # a0 = cv_0 per unit, layout [P, vc]; pre-rotated in packed mode
        a0 = np.zeros((P, 4), dtype=np.float32)
        for vc in range(4):
            v = cvf[e][0, vc * P:(vc + 1) * P]
            if packed:
                a0[:, vc] = np.roll(v.reshape(4, 32), vc, axis=0).reshape(P)
            else:
                a0[:, vc] = v
        in_maps.append({
            "xT": xT,
            "w1": np.asarray(W1[e], np.float32),
            "p5": proj5[e],
            "auxn": auxn.astype(g_np),
            "auxd": auxd.astype(g_np),
            "a0h": a0,
            "b1h": b1h,
            "onesd": np.ones((P, TNMAX), dtype=g_np),
        })
    return in_maps, order, starts, counts, C, b1_zero


def kernel(x, W1, b1, proj, ctrl, scaling, Wg, bg):
    from concourse.bass_utils import run_bass_kernel_spmd

    in_maps, order, starts, counts, C, b1_zero = make_in_maps(
        x, W1, b1, proj, ctrl, scaling, Wg, bg)
    nc, _ = _get_program(C, "f32r", b1_zero)

    res = run_bass_kernel_spmd(nc, in_maps, list(range(N_CORES)))

    out = np.empty((N_TOK, U_DIM), dtype=np.float32)
    for e in range(E_EXP):
        cnt = int(counts[e])
        if cnt:
            out[order[starts[e]:starts[e + 1]]] = \
                res.results[e]["outT"][:, :cnt].T
    return out


MM_MODE = "f32r"  # kept for test.py compatibility
